# revision 22
# baseline (speedup 1.0000x reference)
"""Trainium2 Bass kernel for the temporal/spatial adapter transformer block.

Sharding: data-parallel over the video batch B=8 -> 1 video (16 frames) per
NeuronCore; all weights replicated. No collectives.

Per-core layout strategy:
  - token-major fp32 residual stream (LayerNorm stats via bn_stats,
    batched Rsqrt for rstd),
  - LN gamma/beta folded into consumer weights host-side, so the
    feature-major LN output is a plain transpose (grouped psum->sbuf copies),
  - feature-major bf16 compute stream for all matmul chains,
  - single-op activations: Gelu_apprx_tanh (adapters), Gelu_apprx_sigmoid
    (quickgelu MLP gate),
  - attention with transposed scores (S^T = k^T . q); softmax sums for all
    heads of a sequence accumulate into one 12-row psum tile; 1/sum via the
    fast DVE reciprocal; per-head-pair f32 selector matmul broadcasts the
    recip into the po psum tile's second column block, so normalization is
    one tensor_tensor per 128-feature chunk,
  - S-branch adapter up-projections accumulate directly into the proj/fc2
    psum chains (no separate adapter-output tiles).
"""

import sys

import numpy as np
import ml_dtypes

try:
    import concourse.bass  # noqa: F401
except ImportError:  # concourse ships with the container, not on sys.path
    for p in ("/opt/trn_rl_repo", "/root/.axon_site/_ro/trn_rl_repo"):
        if p not in sys.path:
            sys.path.insert(0, p)

import os

import concourse.bass as bass
import concourse.mybir as mybir
import concourse.tile as tile
from concourse import bacc
from concourse.bass_utils import run_bass_kernel_spmd

BF = mybir.dt.bfloat16
F32 = mybir.dt.float32
AF = mybir.ActivationFunctionType
OP = mybir.AluOpType

# debug substitution: CoreSim lacks Gelu_apprx_*; swap for sim-implemented
# funcs and compare against a numpy model with the same substitution
KDEBUG = bool(os.environ.get("KDEBUG_GELU"))
AF_GELU = AF.Tanh if KDEBUG else AF.Gelu_apprx_tanh
AF_QGELU = AF.Sigmoid if KDEBUG else AF.Gelu_apprx_sigmoid

P = 128
NSEQ = 197          # tokens per frame/sequence
D = 768
DK = D // P         # 6
H = 12
HD = 64
BOT = 192
HID = 4 * D         # 3072
HK = HID // P       # 24
EPS = 1e-5
T = 16              # frames per video
TT = 8              # temporal frames
NCORES = 8
TAU = 2 * NSEQ      # tokens per pair = 394
ROWS = T * NSEQ     # 3152 rows per core

QK_SCALE = HD ** -0.5

ADAPTERS = ("tab", "sa", "ta", "sm", "tm")

bf16 = ml_dtypes.bfloat16


# ----------------------------------------------------------------------------
# host-side weight preprocessing (shared by all cores)
# ----------------------------------------------------------------------------

def preprocess_weights(inp):
    """Build the per-core constant input arrays (already in SBUF layout).

    LayerNorm gammas are folded into the input dims of every LN consumer
    (qkv, tab/sa/sm down-proj, fc1); betas fold into their biases.  The
    temporal branch feeds attention with the tab-adapter OUTPUT, so tab's
    up-proj is divided by gamma1 to cancel the fold inside wqkv.
    """
    w = {}
    f32 = lambda k: np.asarray(inp[k], np.float32)

    g1, b1 = f32("n1_g"), f32("n1_b")
    g2, b2 = f32("n2_g"), f32("n2_b")
    assert np.abs(g1).min() > 1e-3 and np.abs(g2).min() > 1e-3

    def fm(mat):  # [out, in] -> lhsT layout [128, in//128, out]
        o, i = mat.shape
        return np.ascontiguousarray(
            mat.T.reshape(i // P, P, o).transpose(1, 0, 2)).astype(bf16)

    def colmaj(vec):  # [n*128] -> [128, n]
        n = vec.shape[0] // P
        return np.ascontiguousarray(vec.reshape(n, P).T)

    qkv_w = f32("qkv_w")
    bqkv = qkv_w @ b1
    assert np.abs(bqkv[:2 * D]).max() < 1e-6, "nonzero q/k LN beta unsupported"
    qkv = qkv_w * g1[None, :]
    qkv[:D] *= QK_SCALE  # fold attention scale into q
    w["wqkv"] = fm(qkv)                                   # [128, 6, 2304]

    proj_w, proj_b = f32("proj_w"), f32("proj_b")
    w["wproj"] = fm(proj_w)                               # [128, 6, 768]
    w["bproj"] = colmaj(proj_b)                           # T branch
    bproj_s = proj_b + proj_w @ bqkv[2 * D:]              # v-bias folds through attn
    w["bprojsa"] = colmaj(bproj_s + f32("sa_ub"))         # S branch (+ sa up bias)

    fc1_w, fc1_b = f32("fc1_w"), f32("fc1_b")
    a = fm(fc1_w * g2[None, :])                           # [128, 6, 3072]
    w["wfc1"] = np.ascontiguousarray(
        a.reshape(P, DK, HK, P).transpose(2, 0, 1, 3).reshape(HK, P, DK * P))
    w["bfc1"] = colmaj(fc1_b + fc1_w @ b2)                # [128, 24]
    fc2_w, fc2_b = f32("fc2_w"), f32("fc2_b")
    a = fm(fc2_w)                                         # [128, 24, 768]
    w["wfc2"] = np.ascontiguousarray(
        a.reshape(P, HK, DK, P).transpose(2, 0, 1, 3).reshape(DK, P, HK * P))
    w["bfc2"] = colmaj(fc2_b)                             # T branch
    w["bfc2sm"] = colmaj(fc2_b + f32("sm_ub"))            # S branch (+ sm up bias)

    folds = {"tab": (g1, b1), "sa": (g1, b1), "sm": (g2, b2),
             "ta": (None, None), "tm": (None, None)}
    for ad in ADAPTERS:
        dw, db = f32(ad + "_dw"), f32(ad + "_db")
        uw, ub = f32(ad + "_uw"), f32(ad + "_ub")
        g, b = folds[ad]
        if g is not None:
            db = db + dw @ b
            dw = dw * g[None, :]
        if ad == "tab":  # cancel the g1 fold inside wqkv for the T branch
            uw = uw / g1[:, None]
            ub = ub / g1
        w["w%sd" % ad] = fm(dw)                           # [128, 6, 192]
        bd = np.zeros((P, 2), np.float32)
        bd[:, 0] = db[:P]
        bd[:64, 1] = db[P:]
        w["b%sd" % ad] = bd
        up = np.zeros((2 * P, D), np.float32)
        up[:BOT] = uw.T
        w["w%su" % ad] = up.reshape(2, P, D).transpose(1, 0, 2).astype(bf16)
        if ad in ("tab", "ta", "tm"):
            w["b%su" % ad] = colmaj(ub)

    w["ident"] = np.eye(P, dtype=bf16)
    w["ones1"] = np.ones((P, 1), dtype=bf16)
    w["onesb"] = np.ones((P, 64), dtype=bf16)
    w["epsc"] = np.full((P, 1), EPS, np.float32)
    return w


STREAMED_SPECS = [
    ("wfc1", [HK, P, DK * P], BF),
    ("wfc2", [DK, P, HK * P], BF),
]

WEIGHT_SPECS = [
    ("wqkv", [P, DK, 3 * D], BF),
    ("wproj", [P, DK, D], BF),
    ("bproj", [P, DK], F32), ("bprojsa", [P, DK], F32),
    ("bfc1", [P, HK], F32),
    ("bfc2", [P, DK], F32), ("bfc2sm", [P, DK], F32),
    ("ident", [P, P], BF), ("ones1", [P, 1], BF), ("onesb", [P, 64], BF),
    ("epsc", [P, 1], F32),
] + [
    it for ad in ADAPTERS for it in [
        ("w%sd" % ad, [P, DK, BOT], BF),
        ("b%sd" % ad, [P, 2], F32),
        ("w%su" % ad, [P, 2, D], BF),
    ]
] + [("b%su" % ad, [P, DK], F32) for ad in ("tab", "ta", "tm")]


# ----------------------------------------------------------------------------
# program emission
# ----------------------------------------------------------------------------

# token tiles of a pair: (row_offset_within_pair, nrows, fm_col_offset)
PAIR_TILES = [(0, P, 0), (P, NSEQ - P, P),
              (NSEQ, P, NSEQ), (NSEQ + P, NSEQ - P, NSEQ + P)]


class Ctx:
    pass


def make_pools(ctx, tc, es):
    def pool(name, bufs):
        return es.enter_context(tc.tile_pool(name=name, bufs=bufs))

    def ppool(name, bufs):
        return es.enter_context(tc.tile_pool(name=name, bufs=bufs, space="PSUM"))

    ctx.weights = pool("weights", 1)
    ctx.xres = pool("xres", 6)       # token-major f32 residual stream
    ctx.small = pool("small", 8)     # bn stats, mv, rstd
    ctx.xn = pool("xn", 4)           # token-major bf16 LN output
    ctx.fmA = pool("fmA", 3)         # xnT / xn2T
    ctx.fmB = pool("fmB", 2)         # tab-out / attnT / mlpT (matmul inputs)
    ctx.fmC = pool("fmC", 2)         # delta tiles
    ctx.qk = pool("qk", 2)           # q,k feature-major
    ctx.vt = pool("vt", 4)           # v token-major
    ctx.oT = pool("oT", 2)
    ctx.ae = pool("ae", 3)           # exp'd scores bf16
    ctx.rr = pool("rr", 2)           # softmax recip rows f32
    ctx.rrb = pool("rrb", 2)         # recip rows bf16
    ctx.rbs = pool("rbs", 2)         # broadcast recip bf16
    ctx.g2 = pool("g2", 1)           # mlp gelu output
    ctx.wf1 = pool("wf1", 3)         # streamed fc1 weight tiles
    ctx.wf2 = pool("wf2", 2)         # streamed fc2 weight tiles
    ctx.u = pool("u", 4)             # adapter gelu outputs bf16

    ctx.pmm = ppool("pmm", 2)        # dense matmul outputs [128, TAU]
    ctx.ptp = ppool("ptp", 2)        # transpose groups [128, 3, 128]
    ctx.psT = ppool("psT", 1)        # scores
    ctx.po = ppool("po", 2)          # attention o
    ctx.smh = ppool("smh", 1)        # softmax sums + recip broadcast


def load_weights(ctx, nc, d):
    ctx.W = {}
    for name, shape, dt in WEIGHT_SPECS:
        t = ctx.weights.tile(shape, dt, tag=name)
        nc.sync.dma_start(t[:], d[name][:])
        ctx.W[name] = t


def emit_ln(ctx, nc, xts, tiles):
    """token-major LN on xts (f32) -> feature-major bf16 [128, DK, TAU].
    No gamma/beta (folded into consumer weights)."""
    W = ctx.W
    mvs = ctx.small.tile([P, 4, 2], F32, tag="mvs")
    for i, (r0, pi, co) in enumerate(tiles):
        xt = xts[i]
        st = ctx.small.tile([P, 2, 6], F32, tag="bnst")
        nc.vector.bn_stats(st[:pi, 0, :], xt[:pi, 0:D // 2])
        nc.vector.bn_stats(st[:pi, 1, :], xt[:pi, D // 2:D])
        nc.vector.bn_aggr(mvs[:pi, i, :], st[:pi])
    sd = ctx.small.tile([P, 4], F32, tag="sd")
    epsc = ctx.W["epsc"][:, 0:1]
    nc.scalar.activation(sd[:, 0:4:2], mvs[:, 0:4:2, 1], AF.Sqrt, bias=epsc)
    nc.scalar.activation(sd[:NSEQ - P, 1:4:2], mvs[:NSEQ - P, 1:4:2, 1], AF.Sqrt,
                         bias=epsc[:NSEQ - P])
    rstd = ctx.small.tile([P, 4], F32, tag="rstd")
    nc.vector.reciprocal(rstd[:, 0:4:2], sd[:, 0:4:2])
    nc.vector.reciprocal(rstd[:NSEQ - P, 1:4:2], sd[:NSEQ - P, 1:4:2])
    xns = []
    for i, (r0, pi, co) in enumerate(tiles):
        xn = ctx.xn.tile([P, D], BF, tag="xn")
        nc.gpsimd.tensor_scalar(xn[:pi], xts[i][:pi], mvs[:pi, i, 0:1],
                                rstd[:pi, i:i + 1], op0=OP.subtract, op1=OP.mult)
        xns.append(xn)
    xnT = ctx.fmA.tile([P, DK, TAU], BF, tag="xnT")
    for i, (r0, pi, co) in enumerate(tiles):
        for g3 in range(2):
            tp = ctx.ptp.tile([P, 3, P], BF, tag="tp", name="tp")
            for j3 in range(3):
                j = 3 * g3 + j3
                nc.tensor.transpose(tp[:, j3, :pi], xns[i][:pi, j * P:(j + 1) * P],
                                    W["ident"][:pi, :pi])
            nc.vector.tensor_copy(xnT[:, 3 * g3:3 * g3 + 3, co:co + pi],
                                  tp[:, :, :pi])
    return xnT


def emit_adapter_gs(ctx, nc, ad, inT):
    """adapter down-proj + gelu on feature-major input; returns the two
    bf16 gelu chunks [(tile, rows)]."""
    W = ctx.W
    wd, bd = W["w%sd" % ad], W["b%sd" % ad]
    gs = []
    for oc, (ob, osz) in enumerate(((0, P), (P, 64))):
        ps = ctx.pmm.tile([P, 512], F32, tag="mm", name="mmps")
        ps = ps[:, :TAU]
        for k in range(DK):
            nc.tensor.matmul(ps[:osz], wd[:, k, ob:ob + osz], inT[:, k, :],
                             start=(k == 0), stop=(k == DK - 1))
        g = ctx.u.tile([P, TAU], BF, tag="gad%d" % oc)
        nc.scalar.activation(g[:osz], ps[:osz], AF_GELU,
                             bias=bd[:osz, oc:oc + 1])
        gs.append((g, osz))
    return gs


def emit_adapter(ctx, nc, ad, inT, combine):
    """standalone adapter: down -> gelu -> up; combine(mc, psum)."""
    gs = emit_adapter_gs(ctx, nc, ad, inT)
    wu = ctx.W["w%su" % ad]
    for mc in range(DK):
        ps = ctx.pmm.tile([P, 512], F32, tag="mm", name="mmps")
        ps = ps[:, :TAU]
        nc.tensor.matmul(ps[:], wu[:, 0, mc * P:(mc + 1) * P], gs[0][0][:],
                         start=True, stop=False)
        nc.tensor.matmul(ps[:], wu[:64, 1, mc * P:(mc + 1) * P], gs[1][0][:64],
                         start=False, stop=True)
        combine(mc, ps)


def emit_attention(ctx, nc, inT, tiles):
    """multi-head attention core: feature-major input inT (post-LN/adapter).
    Returns oT (feature-major, softmax-normalized, pre-proj)."""
    W = ctx.W
    wq = ctx.W["wqkv"]
    # q,k feature-major
    qkT = ctx.qk.tile([P, 2 * DK, TAU], BF, tag="qkT")
    for oc in range(2 * DK):
        ps = ctx.pmm.tile([P, 512], F32, tag="mm", name="mmps")
        ps = ps[:, :TAU]
        for k in range(DK):
            nc.tensor.matmul(ps[:], wq[:, k, oc * P:(oc + 1) * P], inT[:, k, :],
                             start=(k == 0), stop=(k == DK - 1))
        nc.scalar.copy(qkT[:, oc, :], ps[:])
    # v token-major
    vts = []
    for i, (r0, pi, co) in enumerate(tiles):
        vt = ctx.vt.tile([P, D], BF, tag="vtok")
        for nb, nsz in ((0, 512), (512, 256)):
            ps = ctx.pmm.tile([P, 512], F32, tag="mm", name="psv")
            for k in range(DK):
                nc.tensor.matmul(ps[:pi, :nsz], inT[:, k, co:co + pi],
                                 wq[:, k, 2 * D + nb:2 * D + nb + nsz],
                                 start=(k == 0), stop=(k == DK - 1))
            nc.any.tensor_copy(vt[:pi, nb:nb + nsz], ps[:pi, :nsz])
        vts.append(vt)
    oT = ctx.oT.tile([P, DK, TAU], BF, tag="oT")
    kts = ((0, P), (P, NSEQ - P))
    for j in range(2):  # seq in pair
        c0 = j * NSEQ
        po = r = None
        for h in range(H):
            qof = 64 * (h % 2)
            qch, kch = h // 2, DK + h // 2
            q = qkT[qof:qof + 64, qch, c0:c0 + NSEQ]
            sT = ctx.psT.tile([P, 2, 256], F32, tag="sT", name="sT")
            for kt, (kb, kp) in enumerate(kts):
                nc.tensor.matmul(sT[:kp, kt, :NSEQ],
                                 qkT[qof:qof + 64, kch, c0 + kb:c0 + kb + kp],
                                 q, start=True, stop=True)
            ae = ctx.ae.tile([P, 2, NSEQ], BF, tag="ae")
            nc.scalar.activation(ae[:, 0, :], sT[:, 0, :NSEQ], AF.Exp)
            nc.scalar.activation(ae[:NSEQ - P, 1, :], sT[:NSEQ - P, 1, :NSEQ], AF.Exp)
            if h % 2 == 0:
                po = ctx.po.tile([P, 256], F32, tag="po", name="po")
                smh = ctx.smh.tile([P, 2, 256], F32, tag="smh", name="smh")
                r = ctx.rr.tile([P, 256], F32, tag="r")
                rb = ctx.rrb.tile([P, 256], BF, tag="rb")
            # exp'd scores -> o (po bank) and softmax sum (one row of smh bank)
            for kt, (kb, kp) in enumerate(kts):
                nc.tensor.matmul(po[qof:qof + 64, :NSEQ],
                                 vts[2 * j + kt][:kp, h * HD:(h + 1) * HD],
                                 ae[:kp, kt, :], start=(kt == 0), stop=(kt == 1))
                nc.tensor.matmul(smh[qof:qof + 1, 0, :NSEQ], W["ones1"][:kp, 0:1],
                                 ae[:kp, kt, :], start=(kt == 0), stop=(kt == 1))
            nc.vector.reciprocal(r[qof:qof + 1, :NSEQ],
                                 smh[qof:qof + 1, 0, :NSEQ])
            nc.vector.tensor_copy(rb[qof:qof + 1, :NSEQ], r[qof:qof + 1, :NSEQ])
            # broadcast 1/sum over the head's 64 rows (col block 1 of smh bank)
            nc.tensor.matmul(smh[qof:qof + 64, 1, :NSEQ],
                             W["onesb"][qof:qof + 1, :],
                             rb[qof:qof + 1, :NSEQ], start=True, stop=True)
            if h % 2 == 1:
                c = h // 2
                rbs = ctx.rbs.tile([P, 256], BF, tag="rbs")
                nc.vector.tensor_copy(rbs[:, :NSEQ], smh[:, 1, :NSEQ])
                nc.vector.tensor_tensor(oT[:, c, c0:c0 + NSEQ], po[:, :NSEQ],
                                        rbs[:, :NSEQ], op=OP.mult)
    return oT


def emit_matmul_fm(ctx, nc, wname, kn, inT, combine, post=None):
    """dense feature-major matmul: out[:, mc, :] for mc in range(6).
    post(mc, ps) optionally appends accumulating matmuls (it must stop)."""
    w = ctx.W[wname]
    for mc in range(DK):
        ps = ctx.pmm.tile([P, 512], F32, tag="mm", name="mmps")
        ps = ps[:, :TAU]
        for k in range(kn):
            nc.tensor.matmul(ps[:], w[:, k, mc * P:(mc + 1) * P], inT[:, k, :],
                             start=(k == 0), stop=(post is None and k == kn - 1))
        if post is not None:
            post(mc, ps)
        combine(mc, ps)


def emit_fc2(ctx, nc, d, g2, combine, post=None):
    for mc in range(DK):
        wt = ctx.wf2.tile([P, HK * P], BF, tag="wf2")
        nc.sync.dma_start(wt[:], d["wfc2"][mc])
        ps = ctx.pmm.tile([P, 512], F32, tag="mm", name="mmps")
        ps = ps[:, :TAU]
        for k in range(HK):
            nc.tensor.matmul(ps[:], wt[:, k * P:(k + 1) * P], g2[:, k, :],
                             start=(k == 0), stop=(post is None and k == HK - 1))
        if post is not None:
            post(mc, ps)
        combine(mc, ps)


def adapter_post(ctx, nc, ad, gs):
    """returns post(mc, ps) appending the adapter up-proj accumulation."""
    wu = ctx.W["w%su" % ad]

    def post(mc, ps):
        nc.tensor.matmul(ps[:], wu[:, 0, mc * P:(mc + 1) * P], gs[0][0][:],
                         start=False, stop=False)
        nc.tensor.matmul(ps[:], wu[:64, 1, mc * P:(mc + 1) * P], gs[1][0][:64],
                         start=False, stop=True)
    return post


def emit_delta_add(ctx, nc, deltaT, xts, tiles):
    """transpose feature-major delta and accumulate into token-major xts."""
    W = ctx.W
    for i, (r0, pi, co) in enumerate(tiles):
        for g3 in range(2):
            tp = ctx.ptp.tile([P, 3, P], BF, tag="tp", name="tp")
            for j3 in range(3):
                j = 3 * g3 + j3
                nc.tensor.transpose(tp[:pi, j3, :], deltaT[:, j, co:co + pi],
                                    W["ident"][:, :])
            nc.vector.tensor_tensor(
                xts[i][:pi, g3 * 3 * P:(g3 + 1) * 3 * P],
                xts[i][:pi, g3 * 3 * P:(g3 + 1) * 3 * P],
                tp[:pi, :, :], op=OP.add)


def emit_pair_gen(ctx, nc, d, branch, rowbase):
    W = ctx.W
    tiles = PAIR_TILES
    # ---- stage A: load + LN1
    xts = []
    for (r0, pi, co) in tiles:
        xt = ctx.xres.tile([P, D], F32, tag="xres")
        nc.sync.dma_start(xt[:pi], d["x"][bass.ds(rowbase + r0, pi), :])
        xts.append(xt)
    xnT = emit_ln(ctx, nc, xts, tiles)
    yield

    # ---- branch-specific pre-attention
    if branch == "T":
        aT = ctx.fmB.tile([P, DK, TAU], BF, tag="fmB")

        def tab_comb(mc, ps):
            nc.scalar.activation(aT[:, mc, :], ps[:], AF.Identity,
                                 bias=W["btabu"][:, mc:mc + 1])
        emit_adapter(ctx, nc, "tab", xnT, tab_comb)
        attn_in = aT
        sa_gs = None
    else:
        sa_gs = emit_adapter_gs(ctx, nc, "sa", xnT)
        attn_in = xnT
    yield

    # ---- attention
    oT = emit_attention(ctx, nc, attn_in, tiles)
    yield

    # ---- proj (+ branch combine) -> delta1
    delta1 = ctx.fmC.tile([P, DK, TAU], BF, tag="fmC")
    if branch == "T":
        attnT = ctx.fmB.tile([P, DK, TAU], BF, tag="fmB")

        def proj_comb(mc, ps):
            nc.scalar.activation(attnT[:, mc, :], ps[:], AF.Identity,
                                 bias=W["bproj"][:, mc:mc + 1])
        emit_matmul_fm(ctx, nc, "wproj", DK, oT, proj_comb)

        def ta_comb(mc, ps):
            nc.scalar.activation(delta1[:, mc, :], ps[:], AF.Identity,
                                 bias=W["btau"][:, mc:mc + 1])
        emit_adapter(ctx, nc, "ta", attnT, ta_comb)
    else:
        def proj_comb_s(mc, ps):
            nc.scalar.activation(delta1[:, mc, :], ps[:], AF.Identity,
                                 bias=W["bprojsa"][:, mc:mc + 1])
        emit_matmul_fm(ctx, nc, "wproj", DK, oT, proj_comb_s,
                       post=adapter_post(ctx, nc, "sa", sa_gs))

    # ---- first residual: x2 = x + delta1 (in-place on xts)
    emit_delta_add(ctx, nc, delta1, xts, tiles)
    yield

    # ---- LN2
    xn2T = emit_ln(ctx, nc, xts, tiles)
    yield

    # ---- MLP (+ sm adapter for spatial)
    sm_gs = None
    if branch == "S":
        sm_gs = emit_adapter_gs(ctx, nc, "sm", xn2T)

    g2 = ctx.g2.tile([P, HK, TAU], BF, tag="g2")
    for oc in range(HK):
        wt = ctx.wf1.tile([P, DK * P], BF, tag="wf1")
        nc.sync.dma_start(wt[:], d["wfc1"][oc])
        ps = ctx.pmm.tile([P, 512], F32, tag="mm", name="mmps")
        ps = ps[:, :TAU]
        for k in range(DK):
            nc.tensor.matmul(ps[:], wt[:, k * P:(k + 1) * P],
                             xn2T[:, k, :], start=(k == 0), stop=(k == DK - 1))
        nc.scalar.activation(g2[:, oc, :], ps[:], AF_QGELU,
                             bias=W["bfc1"][:, oc:oc + 1])
    yield

    delta2 = ctx.fmC.tile([P, DK, TAU], BF, tag="fmC")
    if branch == "T":
        mlpT = ctx.fmB.tile([P, DK, TAU], BF, tag="fmB")

        def fc2_comb(mc, ps):
            nc.scalar.activation(mlpT[:, mc, :], ps[:], AF.Identity,
                                 bias=W["bfc2"][:, mc:mc + 1])
        emit_fc2(ctx, nc, d, g2, fc2_comb)

        def tm_comb(mc, ps):
            nc.scalar.activation(delta2[:, mc, :], ps[:], AF.Identity,
                                 bias=W["btmu"][:, mc:mc + 1])
        emit_adapter(ctx, nc, "tm", mlpT, tm_comb)
    else:
        def fc2_comb_s(mc, ps):
            nc.scalar.activation(delta2[:, mc, :], ps[:], AF.Identity,
                                 bias=W["bfc2sm"][:, mc:mc + 1])
        emit_fc2(ctx, nc, d, g2, fc2_comb_s,
                 post=adapter_post(ctx, nc, "sm", sm_gs))

    # ---- second residual + store
    emit_delta_add(ctx, nc, delta2, xts, tiles)
    for i, (r0, pi, co) in enumerate(tiles):
        nc.sync.dma_start(d["y"][bass.ds(rowbase + r0, pi), :], xts[i][:pi, :])


def build_program(npairs=4, loop=True, reps=1):
    import contextlib
    nc = bacc.Bacc("TRN2", target_bir_lowering=False, debug=False,
                   num_devices=NCORES)
    d = {}
    d["x"] = nc.dram_tensor("x", [ROWS, D], F32, kind="ExternalInput").ap()
    for name, shape, dt in WEIGHT_SPECS + STREAMED_SPECS:
        d[name] = nc.dram_tensor(name, shape, dt, kind="ExternalInput").ap()
    d["y"] = nc.dram_tensor("y", [ROWS, D], F32, kind="ExternalOutput").ap()

    with tile.TileContext(nc) as tc:
        with contextlib.ExitStack() as es:
            ctx = Ctx()
            make_pools(ctx, tc, es)
            load_weights(ctx, nc, d)

            def body_pairgroup(i):
                for g in (emit_pair_gen(ctx, nc, d, "T", i),
                          emit_pair_gen(ctx, nc, d, "S", i + TT * NSEQ)):
                    for _ in g:
                        pass

            def body_all():
                if loop:
                    with tc.For_i(0, npairs * TAU, TAU, staggered_reset=True) as i:
                        body_pairgroup(i)
                else:
                    for p in range(npairs):
                        body_pairgroup(p * TAU)

            if reps > 1:
                with tc.For_i(0, reps, 1):
                    body_all()
            else:
                body_all()
    nc.compile()
    return nc


# ----------------------------------------------------------------------------
# harness entry point
# ----------------------------------------------------------------------------

_CACHED = {}


def kernel(**inputs):
    if "nc" not in _CACHED:
        _CACHED["nc"] = build_program()
    nc = _CACHED["nc"]
    w = preprocess_weights(inputs)
    x = np.asarray(inputs["x"], np.float32)  # [128, 197, 768]
    in_maps = []
    for c in range(NCORES):
        m = dict(w)
        m["x"] = np.ascontiguousarray(
            x[c * T:(c + 1) * T].reshape(ROWS, D))
        in_maps.append(m)
    res = run_bass_kernel_spmd(nc, in_maps, core_ids=list(range(NCORES)))
    out = np.stack([r["y"].reshape(T, NSEQ, D) for r in res.results])
    return out.reshape(NCORES * T, NSEQ, D)
